# revision 1
# baseline (speedup 1.0000x reference)
"""Grouped-Query Attention (B=2, T=2048, C=2048, 16 Q heads / 4 KV heads,
D=128) on 8 Trainium2 NeuronCores.

Sharding: core (b, g) for b in {0,1}, g in {0..3} handles batch b and KV head
g (= query heads 4g..4g+3). Each core computes its 4 heads' attention plus the
partial output projection against its 512-row slice of Wo; the host sums the
4 partials per batch (the "all-reduce" of the o_proj, done in numpy).

On-core dataflow (all matmuls in float32r — full-rate fp32 on the PE):
  phase 1:  qT/kT/vT projections, transposed layout [d, t] via
            lhsT=W-tile [c,d-chunk], rhs=xT-tile [c, t-block].
  phase 1.5: RoPE on qT/kT (stream_shuffle pair-swap + cos/sin tables),
             vT -> v natural chunks via TensorE transpose.
  phase 2:  per (head, t-block): scores^T [s, t] = k-chunk^T q, exp on
            ScalarE (scale folded in), optional binary mask multiply,
            denominator via ones-matmul, P@V accumulation -> outT [d, t],
            rescale by 1/denominator (partition_broadcast on GpSimd).
  phase 3:  per t-block: o_proj partial [t, c] accumulated over 4 heads,
            DMA to DRAM.

Masking is specialized at build time from the actual mask input: each
(t-block, s-tile) is classified full / skip / partial (partial tiles get a
host-built 0/1 multiplicative mask applied after exp). mask=all-ones -> no
mask work at all; mask=causal tril -> upper tiles skipped, 4 unique diagonal
mask tiles.
"""
import sys

sys.path.insert(0, "/opt/trn_rl_repo")

import numpy as np

B, T, C = 2, 2048, 2048
NUM_HEADS, NUM_KV_HEADS, HEAD_DIM = 16, 4, 128
G = NUM_HEADS // NUM_KV_HEADS  # 4 query heads per core
SCALE = float(HEAD_DIM) ** -0.5
TB = 512  # t-block (matmul moving free dim)
NTB = T // TB  # 4
ST = 128  # s-tile
NST = T // ST  # 16
NCT = C // 128  # 16 contraction tiles

SWAP_MASK = [i ^ 1 for i in range(32)]

_nc_cache: dict = {}


def _classify_mask(mask2d: np.ndarray):
    """mask2d[t, s] bool. Returns (plan, mask_tiles) where
    plan[tb] = list of (s_tile_idx, mask_id or None) and mask_tiles is a
    float32 array [n, 128, TB] of deduplicated partial-tile masks in the
    transposed [s, t] tile layout."""
    plan = []
    uniq: dict = {}
    tiles = []
    for tb in range(NTB):
        sub_t = mask2d[tb * TB : (tb + 1) * TB]  # [TB, T]
        entries = []
        for s in range(NST):
            sub = sub_t[:, s * ST : (s + 1) * ST]  # [TB(t), ST(s)]
            if sub.all():
                entries.append((s, None))
            elif not sub.any():
                continue
            else:
                tile_m = np.ascontiguousarray(sub.T.astype(np.float32))  # [s, t]
                key = tile_m.tobytes()
                mid = uniq.get(key)
                if mid is None:
                    mid = len(tiles)
                    uniq[key] = mid
                    tiles.append(tile_m)
                entries.append((s, mid))
        plan.append(tuple(entries))
    mask_tiles = (
        np.stack(tiles) if tiles else np.zeros((0, ST, TB), dtype=np.float32)
    )
    return tuple(plan), mask_tiles


def _build(plan, n_masks, use_f32r=True):
    import concourse.bacc as bacc
    import concourse.mybir as mybir
    import concourse.tile as tile
    from concourse.masks import make_identity

    F32 = mybir.dt.float32
    MMDT = mybir.dt.float32r if use_f32r else mybir.dt.float32
    Exp = mybir.ActivationFunctionType.Exp

    nc = bacc.Bacc()

    def mdma(out, in_):
        nc.sync.dma_start(out=out, in_=in_.bitcast(MMDT) if use_f32r else in_)
    xT_d = nc.declare_dram_parameter("xT", [C, T], F32, isOutput=False)
    wq_d = nc.declare_dram_parameter("wq", [C, G * HEAD_DIM], F32, isOutput=False)
    wk_d = nc.declare_dram_parameter("wk", [C, HEAD_DIM], F32, isOutput=False)
    wv_d = nc.declare_dram_parameter("wv", [C, HEAD_DIM], F32, isOutput=False)
    wo_d = nc.declare_dram_parameter("wo", [G * HEAD_DIM, C], F32, isOutput=False)
    on_d = nc.declare_dram_parameter("ones", [128, 1], F32, isOutput=False)
    ct_d = nc.declare_dram_parameter("ctab", [HEAD_DIM, T], F32, isOutput=False)
    st_d = nc.declare_dram_parameter("stab", [HEAD_DIM, T], F32, isOutput=False)
    if n_masks:
        mk_d = nc.declare_dram_parameter(
            "masks", [n_masks * ST, TB], F32, isOutput=False
        )
    out_d = nc.declare_dram_parameter("out", [T, C], F32, isOutput=True)

    with tile.TileContext(nc) as tc:
        const = tc.alloc_tile_pool(name="const", bufs=1)
        qkv = tc.alloc_tile_pool(name="qkv", bufs=1)
        wop = tc.alloc_tile_pool(name="wop", bufs=1)
        outp = tc.alloc_tile_pool(name="outp", bufs=8)

        ones_sb = const.tile([128, 1], MMDT, name="ones_sb")
        mdma(ones_sb, on_d.ap())
        ident = const.tile([128, 128], F32, name="ident")
        make_identity(nc, ident)
        ctab = const.tile([HEAD_DIM, T], F32, name="ctab")
        stab = const.tile([HEAD_DIM, T], F32, name="stab")
        nc.sync.dma_start(out=ctab, in_=ct_d.ap())
        nc.sync.dma_start(out=stab, in_=st_d.ap())
        if n_masks:
            msk_sb = const.tile([ST, n_masks * TB], F32, name="msk_sb")
            for i in range(n_masks):
                nc.sync.dma_start(
                    out=msk_sb[:, i * TB : (i + 1) * TB],
                    in_=mk_d.ap()[i * ST : (i + 1) * ST, :],
                )

        qT = [qkv.tile([128, T], MMDT, name=f"qT{h}") for h in range(G)]
        kT = qkv.tile([128, T], MMDT, name="kT")
        vch = [qkv.tile([128, 128], MMDT, name=f"v{s}") for s in range(NST)]

        wo_sb = [wop.tile([128, C], MMDT, name=f"wo{h}") for h in range(G)]
        for h in range(G):
            mdma(wo_sb[h], wo_d.ap()[h * 128 : (h + 1) * 128, :])

        # ---- phase 1: projections (transposed outputs) ----
        wpool = tc.alloc_tile_pool(name="wpool", bufs=1)
        xs = tc.alloc_tile_pool(name="xs", bufs=4)
        p1ps = tc.alloc_tile_pool(name="p1ps", bufs=1, space="PSUM")

        wq_sb = [wpool.tile([128, G * HEAD_DIM], MMDT, name=f"wq{i}") for i in range(NCT)]
        wk_sb = [wpool.tile([128, HEAD_DIM], MMDT, name=f"wk{i}") for i in range(NCT)]
        wv_sb = [wpool.tile([128, HEAD_DIM], MMDT, name=f"wv{i}") for i in range(NCT)]
        vT = wpool.tile([128, T], F32, name="vT")
        for i in range(NCT):
            sl = slice(i * 128, (i + 1) * 128)
            mdma(wq_sb[i], wq_d.ap()[sl, :])
            mdma(wk_sb[i], wk_d.ap()[sl, :])
            mdma(wv_sb[i], wv_d.ap()[sl, :])

        for tb in range(NTB):
            tsl = slice(tb * TB, (tb + 1) * TB)
            q_ps = [
                p1ps.tile([128, TB], F32, name=f"qps{h}", tag=f"qps{h}")
                for h in range(G)
            ]
            k_ps = p1ps.tile([128, TB], F32, name="kps", tag="kps")
            v_ps = p1ps.tile([128, TB], F32, name="vps", tag="vps")
            for ci in range(NCT):
                xt = xs.tile([128, TB], MMDT, name="xt", tag="xt")
                mdma(xt, xT_d.ap()[ci * 128 : (ci + 1) * 128, tsl])
                first, last = ci == 0, ci == NCT - 1
                for h in range(G):
                    nc.tensor.matmul(
                        q_ps[h],
                        lhsT=wq_sb[ci][:, h * 128 : (h + 1) * 128],
                        rhs=xt,
                        start=first,
                        stop=last,
                    )
                nc.tensor.matmul(
                    k_ps, lhsT=wk_sb[ci], rhs=xt, start=first, stop=last
                )
                nc.tensor.matmul(
                    v_ps, lhsT=wv_sb[ci], rhs=xt, start=first, stop=last
                )
            for h in range(G):
                nc.vector.tensor_copy(qT[h][:, tsl], q_ps[h])
            nc.vector.tensor_copy(kT[:, tsl], k_ps)
            nc.vector.tensor_copy(vT[:, tsl], v_ps)

        # ---- phase 1.5: RoPE on qT/kT; transpose vT -> v natural chunks ----
        rpool = tc.alloc_tile_pool(name="rpool", bufs=3)
        p15ps = tc.alloc_tile_pool(name="p15ps", bufs=2, space="PSUM")
        for src in qT + [kT]:
            for tb in range(NTB):
                tsl = slice(tb * TB, (tb + 1) * TB)
                swp = rpool.tile([128, TB], F32, name="swp", tag="swp")
                tmp = rpool.tile([128, TB], F32, name="tmp", tag="tmp")
                nc.vector.stream_shuffle(swp, src[:, tsl], SWAP_MASK)
                nc.vector.tensor_mul(tmp, src[:, tsl], ctab[:, tsl])
                nc.vector.tensor_mul(swp, swp, stab[:, tsl])
                nc.vector.tensor_add(src[:, tsl], tmp, swp)
        for s in range(NST):
            vtp = p15ps.tile([128, 128], F32, name="vtp", tag="vtp")
            nc.tensor.transpose(vtp, vT[:, s * 128 : (s + 1) * 128], ident)
            nc.vector.tensor_copy(vch[s], vtp)

        rpool.release()
        xs.release()
        wpool.release()
        p15ps.release()
        p1ps.release()

        # ---- phases 2+3 ----
        p2sb = tc.alloc_tile_pool(name="p2sb", bufs=3)
        p2ps = tc.alloc_tile_pool(name="p2ps", bufs=1, space="PSUM")
        p3sb = tc.alloc_tile_pool(name="p3sb", bufs=3)

        for tb in range(NTB):
            tsl = slice(tb * TB, (tb + 1) * TB)
            entries = plan[tb]
            oT_sbs = []
            for h in range(G):
                if not entries:
                    oT_sb = outp.tile([128, TB], MMDT, name="oT", tag="oT")
                    nc.gpsimd.memset(oT_sb, 0.0)
                    oT_sbs.append(oT_sb)
                    continue
                oT_ps = p2ps.tile([128, TB], F32, name="oTps", tag="oTps", bufs=2)
                den = p2ps.tile([1, TB], F32, name="den", tag="den", bufs=2)
                n_e = len(entries)
                for idx, (s, mid) in enumerate(entries):
                    stp = p2ps.tile([128, TB], F32, name="stp", tag="stp", bufs=2)
                    nc.tensor.matmul(
                        stp,
                        lhsT=kT[:, s * 128 : (s + 1) * 128],
                        rhs=qT[h][:, tsl],
                        start=True,
                        stop=True,
                    )
                    ep = p2sb.tile([ST, TB], MMDT, name="ep", tag="ep")
                    nc.scalar.activation(ep, stp, Exp, scale=SCALE)
                    if mid is not None:
                        nc.vector.tensor_mul(
                            ep, ep, msk_sb[:, mid * TB : (mid + 1) * TB]
                        )
                    first, last = idx == 0, idx == n_e - 1
                    nc.tensor.matmul(
                        den, lhsT=ones_sb, rhs=ep, start=first, stop=last
                    )
                    nc.tensor.matmul(
                        oT_ps, lhsT=vch[s], rhs=ep, start=first, stop=last
                    )
                rcp = p2sb.tile([1, TB], F32, name="rcp", tag="rcp")
                nc.vector.reciprocal(rcp, den)
                rb = p2sb.tile([128, TB], F32, name="rb", tag="rb")
                nc.gpsimd.partition_broadcast(rb, rcp)
                oT_sb = outp.tile([128, TB], MMDT, name="oT", tag="oT")
                nc.vector.tensor_mul(oT_sb, oT_ps, rb)
                oT_sbs.append(oT_sb)

            # o_proj partial for this t-block
            for cb in range(C // 512):
                for tch in range(TB // 128):
                    ops = p2ps.tile([128, 512], F32, name="ops", tag="ops", bufs=2)
                    for h in range(G):
                        nc.tensor.matmul(
                            ops,
                            lhsT=oT_sbs[h][:, tch * 128 : (tch + 1) * 128],
                            rhs=wo_sb[h][:, cb * 512 : (cb + 1) * 512],
                            start=h == 0,
                            stop=h == G - 1,
                        )
                    osb = p3sb.tile([128, 512], F32, name="osb", tag="osb")
                    nc.vector.tensor_copy(osb, ops)
                    t0 = tb * TB + tch * 128
                    nc.sync.dma_start(
                        out=out_d.ap()[t0 : t0 + 128, cb * 512 : (cb + 1) * 512],
                        in_=osb,
                    )

        p3sb.release()
        p2sb.release()
        p2ps.release()
        outp.release()
        wop.release()
        qkv.release()
        const.release()

    nc.compile()
    return nc


def _prep_inputs(x, cos, sin, Wq, Wk, Wv, Wo, mask_tiles, n_masks):
    cos = np.asarray(cos, dtype=np.float32).reshape(T, HEAD_DIM // 2)
    sin = np.asarray(sin, dtype=np.float32).reshape(T, HEAD_DIM // 2)
    ctab = np.ascontiguousarray(np.repeat(cos, 2, axis=1).T)  # [128, T]
    s2 = np.repeat(sin, 2, axis=1)
    s2[:, 0::2] *= -1.0
    stab = np.ascontiguousarray(s2.T)

    in_maps = []
    for core in range(8):
        b, g = divmod(core, NUM_KV_HEADS)
        m = {
            "xT": np.ascontiguousarray(np.asarray(x[b], dtype=np.float32).T),
            "wq": np.ascontiguousarray(Wq[:, g * 512 : (g + 1) * 512]).astype(
                np.float32
            ),
            "wk": np.ascontiguousarray(Wk[:, g * 128 : (g + 1) * 128]).astype(
                np.float32
            ),
            "wv": np.ascontiguousarray(Wv[:, g * 128 : (g + 1) * 128]).astype(
                np.float32
            ),
            "wo": np.ascontiguousarray(Wo[g * 512 : (g + 1) * 512, :]).astype(
                np.float32
            ),
            "ctab": ctab,
            "stab": stab,
            "ones": np.ones((128, 1), dtype=np.float32),
        }
        if n_masks:
            m["masks"] = mask_tiles.reshape(n_masks * ST, TB)
        in_maps.append(m)
    return in_maps


def kernel(x, cos, sin, mask, Wq, Wk, Wv, Wo, _trace=False, _result_box=None):
    from concourse.bass_utils import run_bass_kernel_spmd

    mask2d = np.asarray(mask).reshape(T, T).astype(bool)
    plan, mask_tiles = _classify_mask(mask2d)
    n_masks = int(mask_tiles.shape[0])

    use_f32r = True
    key = (plan, n_masks, use_f32r)
    nc = _nc_cache.get(key)
    if nc is None:
        nc = _build(plan, n_masks, use_f32r=use_f32r)
        _nc_cache[key] = nc

    in_maps = _prep_inputs(x, cos, sin, Wq, Wk, Wv, Wo, mask_tiles, n_masks)
    res = run_bass_kernel_spmd(nc, in_maps, core_ids=list(range(8)), trace=_trace)
    if _result_box is not None:
        _result_box.append(res)

    out = np.zeros((B, T, C), dtype=np.float32)
    for core in range(8):
        b = core // NUM_KV_HEADS
        out[b] += res.results[core]["out"]
    return out



# revision 13
# speedup vs baseline: 1.3461x; 1.3461x over previous
"""Grouped-Query Attention (B=2, T=2048, C=2048, 16 Q heads / 4 KV heads,
D=128) on 8 Trainium2 NeuronCores.

Sharding: core (b, g) for b in {0,1}, g in {0..3} handles batch b and KV head
g (= query heads 4g..4g+3). Each core computes its 4 heads' attention plus the
partial output projection against its 512-row slice of Wo; the host sums the
4 partials per batch (the "all-reduce" of the o_proj, done in numpy).

v2 (this file) vs baseline:
  * all matmul operands bf16 (inputs staged bf16 on host); PSUM stays fp32.
  * DMA spread across both HWDGE queues (sync + scalar engines) interleaved
    in consumption order, consts on the gpsimd SWDGE queue; kills the 56us
    startup stall and the single-queue DMA saturation.
  * RoPE runs per-t-block inside phase 1 (overlapped with projections of the
    next block) instead of as a serial phase.
  * softmax denominators batched per t-block: dens copied to one [4,512]
    sbuf tile, ONE reciprocal per t-block (was 16 x 3.3us on DVE), rescale
    via partition-broadcast AP reads (no gpsimd partition_broadcast).
  * o_proj of block tb emitted after scores of block tb' (software pipeline)
    so the PE never waits for the softmax tail.
"""
import sys

sys.path.insert(0, "/opt/trn_rl_repo")

import numpy as np

B, T, C = 2, 2048, 2048
NUM_HEADS, NUM_KV_HEADS, HEAD_DIM = 16, 4, 128
G = NUM_HEADS // NUM_KV_HEADS  # 4 query heads per core
SCALE = float(HEAD_DIM) ** -0.5
TB = 512  # t-block (matmul moving free dim)
NTB = T // TB  # 4
ST = 128  # s-tile
NST = T // ST  # 16
NCT = C // 128  # 16 contraction tiles

SWAP_MASK = [i ^ 1 for i in range(32)]

_nc_cache: dict = {}


def _classify_mask(mask2d: np.ndarray):
    """mask2d[t, s] bool. Returns (plan, mask_tiles) where
    plan[tb] = list of (s_tile_idx, mask_id or None) and mask_tiles is a
    float32 array [n, 128, TB] of deduplicated partial-tile masks in the
    transposed [s, t] tile layout."""
    plan = []
    uniq: dict = {}
    tiles = []
    for tb in range(NTB):
        sub_t = mask2d[tb * TB : (tb + 1) * TB]  # [TB, T]
        entries = []
        for s in range(NST):
            sub = sub_t[:, s * ST : (s + 1) * ST]  # [TB(t), ST(s)]
            if sub.all():
                entries.append((s, None))
            elif not sub.any():
                continue
            else:
                tile_m = np.ascontiguousarray(sub.T.astype(np.float32))  # [s, t]
                key = tile_m.tobytes()
                mid = uniq.get(key)
                if mid is None:
                    mid = len(tiles)
                    uniq[key] = mid
                    tiles.append(tile_m)
                entries.append((s, mid))
        plan.append(tuple(entries))
    mask_tiles = (
        np.stack(tiles) if tiles else np.zeros((0, ST, TB), dtype=np.float32)
    )
    return tuple(plan), mask_tiles


def _build(plan, n_masks):
    import concourse.bacc as bacc
    import concourse.mybir as mybir
    import concourse.tile as tile

    F32 = mybir.dt.float32
    BF16 = mybir.dt.bfloat16
    Exp = mybir.ActivationFunctionType.Exp
    Ln = mybir.ActivationFunctionType.Ln

    nc = bacc.Bacc()

    xT_d = nc.declare_dram_parameter("xT", [C, T], BF16, isOutput=False)
    wq_d = nc.declare_dram_parameter("wq", [C, G * HEAD_DIM], BF16, isOutput=False)
    wk_d = nc.declare_dram_parameter("wk", [C, HEAD_DIM], BF16, isOutput=False)
    wv_d = nc.declare_dram_parameter("wv", [C, HEAD_DIM], BF16, isOutput=False)
    wo_d = nc.declare_dram_parameter("wo", [G * HEAD_DIM, C], BF16, isOutput=False)
    on_d = nc.declare_dram_parameter("ones", [128, 128], BF16, isOutput=False)
    id_d = nc.declare_dram_parameter("ident", [128, 128], BF16, isOutput=False)
    ct_d = nc.declare_dram_parameter("ctab", [HEAD_DIM, T], F32, isOutput=False)
    st_d = nc.declare_dram_parameter("stab", [HEAD_DIM, T], F32, isOutput=False)
    if n_masks:
        mk_d = nc.declare_dram_parameter(
            "masks", [n_masks * ST, TB], BF16, isOutput=False
        )
    out_d = nc.declare_dram_parameter("out", [T, C], BF16, isOutput=True)

    # round-robin between the two HWDGE queues, in consumption order
    hw_eng = [None, None]

    with tile.TileContext(nc) as tc:
        hw_eng[0], hw_eng[1] = nc.sync, nc.scalar

        const = tc.alloc_tile_pool(name="const", bufs=1)
        qkv = tc.alloc_tile_pool(name="qkv", bufs=1)
        wop = tc.alloc_tile_pool(name="wop", bufs=1)

        # --- consts via the gpsimd SWDGE queue (out of the hot queues) ---
        ident = const.tile([128, 128], BF16, name="ident")
        nc.gpsimd.dma_start(out=ident, in_=id_d.ap())
        ctab = const.tile([HEAD_DIM, T], F32, name="ctab")
        stab = const.tile([HEAD_DIM, T], F32, name="stab")
        nc.gpsimd.dma_start(out=ctab, in_=ct_d.ap())
        nc.gpsimd.dma_start(out=stab, in_=st_d.ap())
        ones_sb = const.tile([128, 128], BF16, name="ones_sb")
        nc.gpsimd.dma_start(out=ones_sb, in_=on_d.ap())
        if n_masks:
            msk_sb = const.tile([ST, n_masks * TB], BF16, name="msk_sb")
            for i in range(n_masks):
                nc.gpsimd.dma_start(
                    out=msk_sb[:, i * TB : (i + 1) * TB],
                    in_=mk_d.ap()[i * ST : (i + 1) * ST, :],
                )

        qT = [qkv.tile([128, T], BF16, name=f"qT{h}") for h in range(G)]
        kT = qkv.tile([128, T], BF16, name="kT")
        vT = qkv.tile([128, T], BF16, name="vT")
        vch = [qkv.tile([128, 128], BF16, name=f"v{s}") for s in range(NST)]
        wo_sb = [wop.tile([128, C], BF16, name=f"wo{h}") for h in range(G)]

        # ---- phase 1: projections + RoPE + V transpose, per t-block ----
        wpool = tc.alloc_tile_pool(name="wpool", bufs=1)
        xs = tc.alloc_tile_pool(name="xs", bufs=4)
        rp = tc.alloc_tile_pool(name="rp", bufs=2)
        p1ps = tc.alloc_tile_pool(name="p1ps", bufs=1, space="PSUM")

        wq_sb = [wpool.tile([128, G * HEAD_DIM], BF16, name=f"wq{i}") for i in range(NCT)]
        wk_sb = [wpool.tile([128, HEAD_DIM], BF16, name=f"wk{i}") for i in range(NCT)]
        wv_sb = [wpool.tile([128, HEAD_DIM], BF16, name=f"wv{i}") for i in range(NCT)]

        # interleave weight-chunk and first-block xT loads across both HW
        # queues in the order phase 1 consumes them
        xt0 = []
        for ci in range(NCT):
            sl = slice(ci * 128, (ci + 1) * 128)
            e = hw_eng[ci % 2]
            e.dma_start(out=wq_sb[ci], in_=wq_d.ap()[sl, :])
            e.dma_start(out=wk_sb[ci], in_=wk_d.ap()[sl, :])
            e.dma_start(out=wv_sb[ci], in_=wv_d.ap()[sl, :])
            xt = xs.tile([128, TB], BF16, name="xt", tag="xt")
            hw_eng[(ci + 1) % 2].dma_start(out=xt, in_=xT_d.ap()[sl, 0:TB])
            xt0.append(xt)

        def rope(src_f32, dst_region, tsl):
            swp = rp.tile([128, TB], F32, name="swp", tag="swp")
            t1 = rp.tile([128, TB], F32, name="t1", tag="t1")
            nc.vector.stream_shuffle(swp, src_f32, SWAP_MASK)
            nc.vector.tensor_mul(t1, src_f32, ctab[:, tsl])
            nc.vector.tensor_mul(swp, swp, stab[:, tsl])
            nc.vector.tensor_add(dst_region, t1, swp)

        for tb in range(NTB):
            tsl = slice(tb * TB, (tb + 1) * TB)
            q_ps = [
                p1ps.tile([128, TB], F32, name=f"qps{h}", tag=f"qps{h}")
                for h in range(G)
            ]
            k_ps = p1ps.tile([128, TB], F32, name="kps", tag="kps")
            v_ps = p1ps.tile([128, TB], F32, name="vps", tag="vps")
            for ci in range(NCT):
                if tb == 0:
                    xt = xt0[ci]
                else:
                    xt = xs.tile([128, TB], BF16, name="xt", tag="xt")
                    hw_eng[(ci + tb) % 2].dma_start(
                        out=xt, in_=xT_d.ap()[ci * 128 : (ci + 1) * 128, tsl]
                    )
                first, last = ci == 0, ci == NCT - 1
                for h in range(G):
                    nc.tensor.matmul(
                        q_ps[h],
                        lhsT=wq_sb[ci][:, h * 128 : (h + 1) * 128],
                        rhs=xt,
                        start=first,
                        stop=last,
                    )
                nc.tensor.matmul(
                    k_ps, lhsT=wk_sb[ci], rhs=xt, start=first, stop=last
                )
                nc.tensor.matmul(
                    v_ps, lhsT=wv_sb[ci], rhs=xt, start=first, stop=last
                )
            # drain PSUM via ACT (fp32 staging for rope; V direct to bf16)
            for h in range(G):
                qf = rp.tile([128, TB], F32, name=f"qf{h}", tag=f"qf{h}")
                nc.scalar.copy(qf, q_ps[h])
                rope(qf, qT[h][:, tsl], tsl)
            kf = rp.tile([128, TB], F32, name="kf", tag="kf")
            nc.scalar.copy(kf, k_ps)
            rope(kf, kT[:, tsl], tsl)
            nc.scalar.copy(vT[:, tsl], v_ps)
            # transpose this block's V chunks to natural [s, d] layout
            for sc in range(TB // 128):
                s = tb * (TB // 128) + sc
                vtp = p1ps.tile([128, 128], BF16, name="vtp", tag="vtp")
                nc.tensor.transpose(
                    vtp, vT[:, s * 128 : (s + 1) * 128], ident
                )
                nc.scalar.copy(vch[s], vtp)

        # wo loads after all phase-1 traffic (needed only from o_proj on)
        for h in range(G):
            hw_eng[h % 2].dma_start(
                out=wo_sb[h], in_=wo_d.ap()[h * 128 : (h + 1) * 128, :]
            )

        p1ps.release()
        rp.release()
        xs.release()
        wpool.release()

        # ---- phases 2+3, software-pipelined: o_proj(tb) after scores(tb') ----
        p2sb = tc.alloc_tile_pool(name="p2sb", bufs=3)
        p2ps = tc.alloc_tile_pool(name="p2ps", bufs=1, space="PSUM")
        outp = tc.alloc_tile_pool(name="outp", bufs=2)
        p3sb = tc.alloc_tile_pool(name="p3sb", bufs=4)

        oTu = {}  # (tb) -> list of 4 normalized bf16 tiles
        dma_ctr = [0]
        Div = mybir.AluOpType.divide

        def scores_block(tb):
            tsl = slice(tb * TB, (tb + 1) * TB)
            entries = plan[tb]
            tiles = []
            for h in range(G):
                oTu_t = outp.tile([128, TB], BF16, name=f"oTu{h}", tag=f"oTu{h}")
                tiles.append(oTu_t)
                if not entries:
                    nc.gpsimd.memset(oTu_t, 0.0)
                    continue
                oT_ps = p2ps.tile([128, TB], F32, name="oTps", tag="oTps", bufs=2)
                # den broadcast across all 128 partitions (wide-ones lhsT)
                den = p2ps.tile([128, TB], F32, name="den", tag="den", bufs=2)
                n_e = len(entries)
                for idx, (s, mid) in enumerate(entries):
                    stp = p2ps.tile([128, TB], F32, name="stp", tag="stp", bufs=2)
                    nc.tensor.matmul(
                        stp,
                        lhsT=kT[:, s * 128 : (s + 1) * 128],
                        rhs=qT[h][:, tsl],
                        start=True,
                        stop=True,
                    )
                    ep = p2sb.tile([ST, TB], BF16, name="ep", tag="ep")
                    nc.scalar.activation(ep, stp, Exp, scale=SCALE)
                    if mid is not None:
                        nc.vector.tensor_mul(
                            ep, ep, msk_sb[:, mid * TB : (mid + 1) * TB]
                        )
                    first, last = idx == 0, idx == n_e - 1
                    nc.tensor.matmul(
                        den, lhsT=ones_sb, rhs=ep, start=first, stop=last
                    )
                    nc.tensor.matmul(
                        oT_ps, lhsT=vch[s], rhs=ep, start=first, stop=last
                    )
                # 1/den on ACT as exp(-ln(den)); then one DVE mul normalizes,
                # drains PSUM and casts to bf16
                lnd = p2sb.tile([128, TB], F32, name="lnd", tag="lnd", bufs=2)
                nc.scalar.activation(lnd, den, Ln)
                rcp_b = p2sb.tile([128, TB], F32, name="rcpb", tag="rcpb", bufs=2)
                nc.scalar.activation(rcp_b, lnd, Exp, scale=-1.0)
                nc.vector.tensor_mul(tiles[h], oT_ps, rcp_b)
            oTu[tb] = tiles

        def finish_block(tb):
            tiles = oTu.pop(tb)
            for cb in range(C // 512):
                for tch in range(TB // 128):
                    ops = p2ps.tile([128, 512], F32, name="ops", tag="ops", bufs=2)
                    for h in range(G):
                        nc.tensor.matmul(
                            ops,
                            lhsT=tiles[h][:, tch * 128 : (tch + 1) * 128],
                            rhs=wo_sb[h][:, cb * 512 : (cb + 1) * 512],
                            start=h == 0,
                            stop=h == G - 1,
                        )
                    osb = p3sb.tile([128, 512], BF16, name="osb", tag="osb")
                    nc.vector.tensor_copy(osb, ops)
                    t0 = tb * TB + tch * 128
                    hw_eng[dma_ctr[0] % 2].dma_start(
                        out=out_d.ap()[t0 : t0 + 128, cb * 512 : (cb + 1) * 512],
                        in_=osb,
                    )
                    dma_ctr[0] += 1

        order = [NTB - 1 - i for i in range(NTB)]  # big blocks first
        prev = None
        for tb in order:
            scores_block(tb)
            if prev is not None:
                finish_block(prev)
            prev = tb
        finish_block(prev)

        p3sb.release()
        outp.release()
        p2ps.release()
        p2sb.release()
        wop.release()
        qkv.release()
        const.release()

    nc.compile()
    return nc


def _to_bf16(a):
    import ml_dtypes

    return np.ascontiguousarray(np.asarray(a, dtype=np.float32)).astype(
        ml_dtypes.bfloat16
    )


def _prep_inputs(x, cos, sin, Wq, Wk, Wv, Wo, mask_tiles, n_masks):
    cos = np.asarray(cos, dtype=np.float32).reshape(T, HEAD_DIM // 2)
    sin = np.asarray(sin, dtype=np.float32).reshape(T, HEAD_DIM // 2)
    ctab = np.ascontiguousarray(np.repeat(cos, 2, axis=1).T)  # [128, T]
    s2 = np.repeat(sin, 2, axis=1)
    s2[:, 0::2] *= -1.0
    stab = np.ascontiguousarray(s2.T)

    xTb = [_to_bf16(np.asarray(x[b], dtype=np.float32).T) for b in range(B)]
    in_maps = []
    for core in range(8):
        b, g = divmod(core, NUM_KV_HEADS)
        m = {
            "xT": xTb[b],
            "wq": _to_bf16(Wq[:, g * 512 : (g + 1) * 512]),
            "wk": _to_bf16(Wk[:, g * 128 : (g + 1) * 128]),
            "wv": _to_bf16(Wv[:, g * 128 : (g + 1) * 128]),
            "wo": _to_bf16(Wo[g * 512 : (g + 1) * 512, :]),
            "ctab": ctab,
            "stab": stab,
            "ones": _to_bf16(np.ones((128, 128), dtype=np.float32)),
            "ident": _to_bf16(np.eye(128, dtype=np.float32)),
        }
        if n_masks:
            m["masks"] = _to_bf16(mask_tiles.reshape(n_masks * ST, TB))
        in_maps.append(m)
    return in_maps


def kernel(x, cos, sin, mask, Wq, Wk, Wv, Wo, _trace=False, _result_box=None):
    from concourse.bass_utils import run_bass_kernel_spmd

    mask2d = np.asarray(mask).reshape(T, T).astype(bool)
    plan, mask_tiles = _classify_mask(mask2d)
    n_masks = int(mask_tiles.shape[0])

    key = (plan, n_masks)
    nc = _nc_cache.get(key)
    if nc is None:
        nc = _build(plan, n_masks)
        _nc_cache[key] = nc

    in_maps = _prep_inputs(x, cos, sin, Wq, Wk, Wv, Wo, mask_tiles, n_masks)
    res = run_bass_kernel_spmd(nc, in_maps, core_ids=list(range(8)), trace=_trace)
    if _result_box is not None:
        _result_box.append(res)

    out = np.zeros((B, T, C), dtype=np.float32)
    for core in range(8):
        b = core // NUM_KV_HEADS
        out[b] += np.asarray(res.results[core]["out"], dtype=np.float32)
    return out
